# revision 61
# baseline (speedup 1.0000x reference)
"""Trainium2 Bass kernel for GridSmoother: per-batch SPD grid-Laplacian solve.

System: L = I + Dx^T Wx Dx + Dy^T Wy Dy over a 48x64 grid, solved for 16
channels per batch, B=4 batches.  With weights uniform in [0,1), Gershgorin
gives lambda(L) in [1, 9] -- tiny condition number, so a fixed-coefficient
Chebyshev iteration on the 5-point stencil converges at ~0.5x error per
iteration (14 iterations -> ~1e-4 relative error, tolerance is 2e-2).

Sharding: batch b -> cores {2b, 2b+1}, each core owns 8 channels.
Per-core data layout (SBUF tile [128 partitions, 258 free]):
  partition p = (c_local//4)*64 + h      (c_hi in {0,1}, h in 0..47;
                                          partitions 48-63/112-127 unused)
  free      f = 1 + (c_local%4)*64 + w   (c_lo in {0..3}, w in 0..63)
  f=0 and f=257 are zero guard columns.
Horizontal (w+-1) neighbor access = free-dim offset reads (guards + zeroed
boundary weights make inter-block wraps harmless).  Vertical (h+-1) =
partition shifts done on the TensorEngine with block-diagonal +-1 shift
matrices, accumulated in PSUM together with the diagonal and horizontal
terms (5 matmuls -> P = A*u).

All layout prep happens ON DEVICE from the raw inputs (ae shard [8,48,64] +
wxwy plane [2,48,64] per core) via row-contiguous DMAs; the weight planes
(wxz / wxzUP / wyz / wyzUP) are materialized purely by where the DMAs land
(offset partitions/free columns realize the +-1 shifts, un-written rows
realize the zeroed boundary weights) and diag = 1 + sum of the four planes
is computed by the vector engine.  The shift-matrix constant is uploaded
once per process and kept device-resident; the compiled executable is
cached, so a steady-state call ships only ~1 MB of raw inputs and gets
back the 786 KB output.
"""

import sys

import numpy as np

sys.path.insert(0, "/opt/trn_rl_repo")

import jax
import concourse.bass as bass
from concourse import mybir
from concourse.bass2jax import (
    _bass_exec_p,
    install_neuronx_cc_hook,
    partition_id_tensor,
)
from jax.experimental.shard_map import shard_map
from jax.sharding import Mesh, NamedSharding, PartitionSpec

B, C, H, W = 4, 16, 48, 64
NCORE = 8
CPC = C // 2          # channels per core = 8
FD = 258              # free dim incl. 2 guards
FDA = 256             # active free size
NBLK = 5              # wxz, wxzUP, wyz, wyzUP, diag

F32 = mybir.dt.float32
F16 = mybir.dt.float16

LAM_MAX = 9.0         # Gershgorin bound: 1 + 2*(sum of 4 weights), w in [0,1)
N_ITER = 12

N_DMA_IO = 1 + 2                # smats + btile
N_DMA_W = 8                     # weight planes (broadcast)
IO_TGT = 16 * N_DMA_IO
W_TGT = 16 * N_DMA_W
ALL_TGT = IO_TGT + 16 * 2       # io_sem also counts the output stores


def _shift_mats():
    """lhsT matrices [128,128]: I(+1), Sup(-1 at k=m-1), Sdn(-1 at k=m+1),
    IN(-I).  Shifts act within the two active 48-partition blocks (at
    partition offsets 0 and 64)."""
    ipos = np.eye(128, dtype=np.float32)
    sup = np.zeros((128, 128), dtype=np.float32)
    sdn = np.zeros((128, 128), dtype=np.float32)
    for m in range(128):
        mm = m % 64
        if mm < 48:
            if mm != 0:
                sup[m - 1, m] = -1.0
            if mm != 47:
                sdn[m + 1, m] = -1.0
    ineg = -ipos
    return np.concatenate([ipos, sup, sdn, ineg], axis=1)  # [128, 512]


def _cheby_coeffs(lam_max, n_iter):
    """Returns per-iteration (gamma_k, c_next_k) for the scaled-direction
    Chebyshev recurrence:
        x += gamma_k * u ; r -= gamma_k * A u ; u = c_{k+1} * u + r
    """
    lmin = 1.0
    theta = (lam_max + lmin) / 2.0
    delta = (lam_max - lmin) / 2.0
    sigma1 = theta / delta
    gammas, cnexts = [], []
    gamma = 1.0 / theta
    rho = 1.0 / sigma1
    for _ in range(n_iter):
        rho_next = 1.0 / (2.0 * sigma1 - rho)
        c_next = rho * gamma * delta / 2.0
        gamma_next = 2.0 * rho_next / delta
        gammas.append(gamma)
        cnexts.append(c_next)
        rho, gamma = rho_next, gamma_next
    return gammas, cnexts


def _build():
    """Raw Bass program (explicit semaphores; every instruction carries at
    most one wait -- the walrus codegen on this path rejects multi-wait
    sync_info)."""
    nc = bass.Bass("TRN2", target_bir_lowering=False, debug=False,
                   num_devices=NCORE, detect_race_conditions=False)
    # fp16 wire format for the per-call tensor: halves tunnel traffic;
    # converted to f32 on device (weights/rhs/outputs all fit fp16's 2.4e-4
    # relative precision comfortably within the 2e-2 tolerance).  One merged
    # input tensor per core: channels 0-7 = ae shard, 8 = wx, 9 = wy.
    aep_d = nc.dram_tensor("aep", [CPC + 2, H, W], F16,
                           kind="ExternalInput").ap()
    # fp16 shift matrices: entries are exactly 0/+-1, so no precision loss,
    # and fp16 rhs/lhsT doubles TensorEngine throughput vs f32
    smats_d = nc.dram_tensor("smats", [128, 512], F16,
                             kind="ExternalInput").ap()
    out_d = nc.dram_tensor("out", [CPC, H, W], F16, kind="ExternalOutput").ap()

    gammas, cnexts = _cheby_coeffs(LAM_MAX, N_ITER)
    theta = (LAM_MAX + 1.0) / 2.0

    smats = nc.alloc_sbuf_tensor("smats_s", [128, 512], F16).ap()
    # weights stay fp16 end-to-end: the wire format already rounded them,
    # so fp16 products lose nothing vs converting to f32 first.  u and r are
    # fp16 too: with every non-scalar operand 2-byte/packed/SBUF, the DVE's
    # 2x/4x perf modes kick in on the big elementwise ops (fp16 is
    # scale-free, so the decaying residual keeps full relative precision).
    # x stays f32: it accumulates to full magnitude across iterations.
    wcat = nc.alloc_sbuf_tensor("wcat_s", [128, NBLK * FD], F16).ap()
    xh = nc.alloc_sbuf_tensor("xh_s", [128, FDA], F16).ap()
    r = nc.alloc_sbuf_tensor("r_s", [128, FD], F16).ap()
    u = nc.alloc_sbuf_tensor("u_s", [128, FD], F16).ap()
    x = nc.alloc_sbuf_tensor("x_s", [128, FD], F32).ap()
    # the rhs loads land directly in r's active region -- no staging copy
    btile = r[:, 1:257]
    # pc in fp16: the wcat*u products round to 2.4e-4 relative, accumulated
    # in f32 PSUM -- fp16 rhs keeps the TensorEngine at full rate
    pc = nc.alloc_sbuf_tensor("pc_s", [128, NBLK * FD], F16).ap()
    P = nc.alloc_psum_tensor("P_s", [128, FDA], F32).ap()

    mI = smats[:, 0:128]
    mSup = smats[:, 128:256]
    mSdn = smats[:, 256:384]
    mIN = smats[:, 384:512]

    io_sem = nc.alloc_semaphore("io_sem")     # smats/btile loads + out stores
    w_sem = nc.alloc_semaphore("w_sem")       # weight-plane loads
    dve_sem = nc.alloc_semaphore("dve_sem")   # counts pc-ready TTs
    pe_sem = nc.alloc_semaphore("pe_sem")     # counts matmuls
    prep_sem = nc.alloc_semaphore("prep_sem")  # btile/wcat memsets done
    out_sem = nc.alloc_semaphore("out_sem")   # final xh ready

    with nc.Block() as block:

        # DMA issue is split across the two idle HWDGE queues (SP + Act) so
        # the load phase overlaps.  All loads into memset tiles are gated on
        # prep_sem: queue order alone does NOT make a DMA transfer
        # happen-after an engine op.
        wx = aep_d[CPC]       # [48(h), 64(w)]
        wy = aep_d[CPC + 1]
        # 4x c_lo replication rides a stride-0 source dim (one DMA per
        # plane per partition half)
        wx63 = wx[:, 0:63].rearrange(
            "h (o w) -> h o w", o=1).broadcast_to([48, 4, 63])
        wy47 = wy[0:47, :].rearrange(
            "h (o w) -> h o w", o=1).broadcast_to([47, 4, 64])

        def _weight_loads(eng, chi):
            p0 = chi * 64
            eng.dma_start(
                wcat[p0:p0 + 48, 0 * FD + 1:0 * FD + 257].rearrange(
                    "p (clo w) -> p clo w", w=64)[:, :, 0:63],
                wx63).then_inc(w_sem, 16)
            eng.dma_start(
                wcat[p0:p0 + 48, 1 * FD + 2:1 * FD + 258].rearrange(
                    "p (clo w) -> p clo w", w=64)[:, :, 0:63],
                wx63).then_inc(w_sem, 16)
            eng.dma_start(
                wcat[p0:p0 + 47, 2 * FD + 1:2 * FD + 257].rearrange(
                    "p (clo w) -> p clo w", w=64),
                wy47).then_inc(w_sem, 16)
            eng.dma_start(
                wcat[p0 + 1:p0 + 48, 3 * FD + 1:3 * FD + 257].rearrange(
                    "p (clo w) -> p clo w", w=64),
                wy47).then_inc(w_sem, 16)

        @block.sync
        def _(sp):
            sp.dma_start(smats, smats_d).then_inc(io_sem, 16)
            sp.wait_ge(prep_sem, 2)   # btile + wcat memsets done
            _weight_loads(sp, 0)
            # b tiles: 4 channels per DMA, rows stay contiguous
            for chi in (0, 1):
                p0 = chi * 64
                sp.dma_start(
                    btile[p0:p0 + 48, 0:256].rearrange(
                        "h (clo w) -> h clo w", clo=4),
                    aep_d[chi * 4:(chi + 1) * 4].rearrange(
                        "clo h w -> h clo w"),
                ).then_inc(io_sem, 16)

        @block.scalar
        def _(act):
            act.wait_ge(prep_sem, 2)  # btile + wcat memsets done
            _weight_loads(act, 1)
            act.wait_ge(out_sem, 1)
            for chi in (0, 1):
                p0 = chi * 64
                act.dma_start(
                    out_d[chi * 4:(chi + 1) * 4].rearrange(
                        "clo h w -> h clo w"),
                    xh[p0:p0 + 48, :].rearrange(
                        "h (clo w) -> h clo w", clo=4),
                ).then_inc(io_sem, 16)
            act.wait_ge(io_sem, ALL_TGT)

        @block.tensor
        def _(pe):
            pe.wait_ge(io_sem, IO_TGT)  # smats + btile loaded
            for k in range(N_ITER - 1):
                pe.wait_ge(dve_sem, 2 * k + 1)
                # h-1 / h+1 / diag terms first (3-block TT lands first), so
                # only two matmuls trail the second TT -- shorter chain tail
                pe.matmul(P, mSup, pc[:, 2 * FD + 1:2 * FD + 257],
                          start=True, stop=False).then_inc(pe_sem, 1)
                pe.matmul(P, mSdn, pc[:, 3 * FD + 1:3 * FD + 257],
                          start=False, stop=False).then_inc(pe_sem, 1)
                pe.matmul(P, mI, pc[:, 4 * FD + 1:4 * FD + 257],
                          start=False, stop=False).then_inc(pe_sem, 1)
                pe.wait_ge(dve_sem, 2 * k + 2)
                # w-1 / w+1 terms: free-dim-shifted reads of wxz*u, wxzUP*u
                pe.matmul(P, mIN, pc[:, 0 * FD + 0:0 * FD + 256],
                          start=False, stop=False).then_inc(pe_sem, 1)
                pe.matmul(P, mIN, pc[:, 1 * FD + 2:1 * FD + 258],
                          start=False, stop=True).then_inc(pe_sem, 1)

        @block.vector
        def _(v):
            v.memset(r, 0.0).then_inc(prep_sem, 1)
            v.memset(wcat, 0.0).then_inc(prep_sem, 1)
            v.memset(x, 0.0)
            v.wait_ge(w_sem, W_TGT)   # weight planes loaded
            # diag = 1 + wxz + wxzUP + wyz + wyzUP (guard columns end up at
            # 1.0, harmless: the diag block's guards are never read);
            # overlaps with the smats/btile transfers still in flight
            dg = wcat[:, 4 * FD:5 * FD]
            v.tensor_tensor(dg, wcat[:, 0 * FD:1 * FD],
                            wcat[:, 1 * FD:2 * FD], mybir.AluOpType.add)
            v.tensor_tensor(dg, dg, wcat[:, 2 * FD:3 * FD],
                            mybir.AluOpType.add)
            v.tensor_tensor(dg, dg, wcat[:, 3 * FD:4 * FD],
                            mybir.AluOpType.add)
            v.tensor_scalar_add(dg, dg, 1.0)
            v.wait_ge(io_sem, IO_TGT)  # rhs loaded (directly into r)
            v.tensor_scalar_mul(u, r, 1.0 / theta)
            u_b2 = u.rearrange("p (o f) -> p o f", o=1).broadcast_to(
                [128, 2, FD])
            u_b3 = u.rearrange("p (o f) -> p o f", o=1).broadcast_to(
                [128, 3, FD])
            for k in range(N_ITER):
                g = float(gammas[k])
                if k == N_ITER - 1:
                    # final x-update fused with the fp16 output conversion
                    v.scalar_tensor_tensor(
                        xh, u[:, 1:257], g, x[:, 1:257],
                        mybir.AluOpType.mult,
                        mybir.AluOpType.add).then_inc(out_sem, 1)
                    break
                c = float(cnexts[k])
                v.tensor_tensor(
                    pc[:, 2 * FD:5 * FD].rearrange("p (o f) -> p o f", o=3),
                    wcat[:, 2 * FD:5 * FD].rearrange("p (o f) -> p o f", o=3),
                    u_b3, mybir.AluOpType.mult).then_inc(dve_sem, 1)
                v.tensor_tensor(
                    pc[:, 0:2 * FD].rearrange("p (o f) -> p o f", o=2),
                    wcat[:, 0:2 * FD].rearrange("p (o f) -> p o f", o=2),
                    u_b2, mybir.AluOpType.mult).then_inc(dve_sem, 1)
                # x += gamma * u (hidden in the PE-matmul bubble)
                v.scalar_tensor_tensor(x, u, g, x,
                                       mybir.AluOpType.mult,
                                       mybir.AluOpType.add)
                v.wait_ge(pe_sem, 5 * (k + 1))
                # r -= gamma * P
                v.scalar_tensor_tensor(r[:, 1:257], P, -g, r[:, 1:257],
                                       mybir.AluOpType.mult,
                                       mybir.AluOpType.add)
                # u = c_next * u + r
                v.scalar_tensor_tensor(u, u, c, r,
                                       mybir.AluOpType.mult,
                                       mybir.AluOpType.add)

    return nc


_SESSION = None


def _get_session():
    """Compile once per process; returns (jitted_fn, device-resident smats)."""
    global _SESSION
    if _SESSION is not None:
        return _SESSION

    install_neuronx_cc_hook()
    nc = _build()

    assert nc.dbg_addr is None
    partition_name = (nc.partition_id_tensor.name
                      if nc.partition_id_tensor else None)

    in_names, out_names, out_avals = [], [], []
    for alloc in nc.m.functions[0].allocations:
        if not isinstance(alloc, mybir.MemoryLocationSet):
            continue
        name = alloc.memorylocations[0].name
        if alloc.kind == "ExternalInput":
            if name != partition_name:
                in_names.append(name)
        elif alloc.kind == "ExternalOutput":
            out_names.append(name)
            out_avals.append(jax.core.ShapedArray(
                tuple(alloc.tensor_shape), mybir.dt.np(alloc.dtype)))
    assert in_names == ["aep", "smats"], in_names
    assert out_names == ["out"], out_names
    in_names_all = list(in_names)
    if partition_name is not None:
        in_names_all.append(partition_name)

    def _body(aep, smats):
        operands = [aep, smats]
        if partition_name is not None:
            operands.append(partition_id_tensor())
        outs = _bass_exec_p.bind(
            *operands,
            out_avals=tuple(out_avals),
            in_names=tuple(in_names_all),
            out_names=tuple(out_names),
            lowering_input_output_aliases=(),
            sim_require_finite=True,
            sim_require_nnan=True,
            nc=nc,
        )
        return outs[0]

    devices = jax.devices()[:NCORE]
    assert len(devices) == NCORE
    mesh = Mesh(np.asarray(devices), ("core",))
    spec = PartitionSpec("core")
    fn = jax.jit(shard_map(
        _body, mesh=mesh, in_specs=(spec, spec), out_specs=spec,
        check_rep=False))

    smats_np = np.tile(_shift_mats(), (NCORE, 1)).astype(np.float16)
    smats_dev = jax.device_put(smats_np, NamedSharding(mesh, spec))
    jax.block_until_ready(smats_dev)

    # AOT-compile for the fixed shapes (skips jit-cache machinery per call)
    aep_aval = jax.ShapeDtypeStruct((NCORE * (CPC + 2), H, W), np.float16)
    compiled = fn.lower(aep_aval, smats_dev).compile()

    _SESSION = (compiled, smats_dev)
    return _SESSION


def kernel(ae: np.ndarray, wxwy: np.ndarray) -> np.ndarray:
    ae = np.ascontiguousarray(ae, dtype=np.float32)
    wxwy = np.ascontiguousarray(wxwy, dtype=np.float32)
    assert ae.shape == (B, C, H, W) and wxwy.shape == (B, 2, H, W)

    fn, smats_dev = _get_session()

    # core 2b+half owns channels [half*8, (half+1)*8) of batch b, which is
    # exactly row-major order of (b, c).  Merged upload: per core 8 ae
    # channels + the batch's wx, wy planes as channels 8-9.
    m = np.empty((NCORE, CPC + 2, H, W), np.float16)
    m[:, :CPC] = ae.reshape(NCORE, CPC, H, W)
    m[:, CPC:] = np.repeat(wxwy, 2, axis=0)
    aep_g = m.reshape(NCORE * (CPC + 2), H, W)

    try:
        out = np.asarray(fn(aep_g, smats_dev))
    except Exception:
        # transient tunnel/runtime hiccup: one retry
        out = np.asarray(fn(aep_g, smats_dev))
    return out.astype(np.float32).reshape(B, C, H, W)
